# revision 17
# baseline (speedup 1.0000x reference)
"""Trainium2 Bass kernel for MinibatchDiscrimination (screening formulation).

Reference computation:
    M = (x @ T.reshape(2048, 4096)).reshape(256, 128, 32)       # "matrices"
    norm1[i,j,f] = sum_k |M[i,f,k] - M[j,f,k]|                   (L1 over k)
    o_b[j,f]    = sum_i exp(-norm1[i,j,f])
    out         = concat([x, o_b], axis=1)                       # [256, 2176]

Key observation: in f32, exp(-z) is exactly 0.0 for z > 104 (the result
is below the smallest subnormal). For this problem M has std ~45, so
pairwise L1 norms are ~1600 and every off-diagonal exp underflows to an
exact 0 in the f32 reference; o_b is exactly ones + corrections from
any "close" pair. The kernel therefore SCREENS: it lower-bounds every
off-diagonal L1 norm with the pairwise L2 norm (norm1 >= norm2), which
is computable as per-feature gram matrices on the tensor engine at ~50x
the throughput of the elementwise L1 pass. Rows whose bound cannot
certify underflow are recomputed exactly on the host (f32, matching the
reference); for generic inputs nothing is flagged.

Sharding: OUT_FEATURES (128) split across 8 cores (16 features each),
no collectives, no duplicated matmul work.

Device computation per core:
  M^T tiles [128 fk', 256 i] (fk' = f*32+k, bf16) via PE;
  r[f,i] = sum_k M[i,f,k]^2 via DVE squares + fp32 ones matmul;
  per feature f: PSUM bank holds
      P[i,j] = G_ij - r_i/2 - r_j/2  (= -norm2^2/2)
  from a 32-contraction gram matmul plus a rank-2 fp16 fixup matmul
  ((-r/16) x 8 + 8 x (-r/16)); the h=1 half computes only the (B1,B1)
  block (the (B1,B0) block is the transpose of (B0,B1), already
  covered), so the bank is [128, 384].

Certificate (relu-sum, no diagonal mask):
    cert[p,f] = sum_j relu(P + TH/2) over the bank row p
  Healthy: every off-diagonal P <= -TH/2 so only the two diagonal
  entries contribute: cert = 2*relu(TH/2 + delta) = TH + O(delta),
  where |delta| <= ~128 (fp16 r/16 rounding). Host accepts iff
  |cert - TH| <= BAND.
  Soundness (TH=17800, BAND=400): acceptance implies the total hidden
  relu leak is <= BAND + 2*|delta| + rounding ~= 700, so every
  off-diag P <= -TH/2 + 700 => device-norm2^2 >= TH - 1400 = 16400 =>
  bf16-L2 >= 128 => true f32 L2 >= 116 (bf16-M error <= ~1/entry,
  ||dd||_2 <= 12) => true L1 >= 116 > 104 => reference entry is
  exactly 0. Actual min off-diag norm2^2 is ~18666 > TH, so no leaks
  and no flags on generic inputs; any flag falls back to an exact
  host recompute (still correct, just slow).
  ACT banks use activation(Relu, bias=TH/2, accum_out); DVE banks use
  tensor_scalar(add TH/2, max 0) + tensor_reduce(add). Both healthy
  values are TH + O(delta) (DVE adds bf16 rounding of the two ~TH/2
  diagonal relus, ~ +-64, inside BAND).
"""

import sys

if "/opt/trn_rl_repo" not in sys.path:
    sys.path.insert(0, "/opt/trn_rl_repo")

import ml_dtypes
import numpy as np

import concourse.bacc as bacc
import concourse.bass as bass
import concourse.mybir as mybir
import concourse.tile as tile
from concourse.bass_utils import run_bass_kernel_spmd

N = 256
IN_F = 2048
OUT_F = 128
KD = 32
NCORES = 8
F_LOC = OUT_F // NCORES        # 16 features per core
FK = F_LOC * KD                # 512 (f-major: fk = f*32 + k)
NT = FK // 128                 # 4 fk tiles of 128 partitions (4 features each)
NCT = IN_F // 128              # 16 contraction tiles
NQ = 4                         # input DMA chunks (4 ct each)

TH = 17800.0                   # norm2^2 certification threshold
BAND = 400.0                   # |cert - TH| acceptance band
BANK_W = 3 * 128               # 256 (h0 full) + 128 (h1 B1xB1 block)

F32 = mybir.dt.float32
BF16 = mybir.dt.bfloat16
FP16 = mybir.dt.float16

_CACHE = {}


def _build():
    nc = bacc.Bacc()
    xq_d = [nc.dram_tensor(f"xq{q}", [128, 4 * N], BF16, kind="ExternalInput")
            for q in range(NQ)]
    tq_d = [nc.dram_tensor(f"tq{q}", [128, 4 * FK], BF16, kind="ExternalInput")
            for q in range(NQ)]
    Eig_d = nc.dram_tensor("Eights", [1, F_LOC * N], FP16, kind="ExternalInput")
    Ofp_d = nc.dram_tensor("Ofp", [128, NT], F32, kind="ExternalInput")
    certA_d = nc.dram_tensor("certA", [128, F_LOC], F32, kind="ExternalOutput")
    certD_d = nc.dram_tensor("certD", [128, F_LOC], F32, kind="ExternalOutput")

    with tile.TileContext(nc) as tc:
        with (
            tc.tile_pool(name="persist", bufs=1) as pp,
            tc.tile_pool(name="scr", bufs=4) as scp,
        ):
            # ---- input DMAs: packed fat rows, M inputs first ----
            xq = []
            tq = []
            for q in range(NQ):
                xs = pp.tile([128, 4 * N], BF16, tag=f"xq{q}")
                eng = nc.sync if q < 2 else nc.scalar
                eng.dma_start(xs[:], xq_d[q][:])
                xq.append(xs)
                ts = pp.tile([128, 4 * FK], BF16, tag=f"tq{q}")
                eng.dma_start(ts[:], tq_d[q][:])
                tq.append(ts)

            # fixup operands, co-located with the gram's PE row tile
            # (partitions 32a, 32a+1): different PE row tiles must not
            # accumulate into the same PSUM bank.
            # RFa rows: 32a = -r/16 (stationary), 32a+1 = 8.0
            # RFb rows: 32a = 8.0 (moving),       32a+1 = -r/16
            RFa = pp.tile([98, F_LOC * N], FP16, tag="RFa")
            RFb = pp.tile([98, F_LOC * N], FP16, tag="RFb")
            for a in range(NT):
                nc.scalar.dma_start(RFa[32 * a + 1:32 * a + 2, :], Eig_d[:])
                nc.scalar.dma_start(RFb[32 * a:32 * a + 1, :], Eig_d[:])
            Ofp = pp.tile([128, NT], F32, tag="Ofp")
            nc.scalar.dma_start(Ofp[:], Ofp_d[:])

            # separate per-engine cert tiles: a shared tile would make
            # Tile serialize the ACT and DVE drains against each other
            certA = pp.tile([128, F_LOC], F32, tag="certA")
            nc.vector.memset(certA[:], 0.0)
            certD = pp.tile([128, F_LOC], F32, tag="certD")
            nc.vector.memset(certD[:], 0.0)
            bth = pp.tile([128, 1], F32, tag="bth")
            nc.vector.memset(bth[:], TH / 2)
            rb = pp.tile([100, N], FP16, tag="rb")

            # ---- phase 1: M^T tiles [128 fk', 256 i], ct-outer so the
            # tensor engine starts after the first input chunk; the PSUM
            # pool closes before phase 2 so the gram pool gets 6 banks ----
            Mts = []
            with tc.tile_pool(name="mpsum", bufs=1,
                              space=bass.MemorySpace.PSUM) as mpp:
                # r rows live at partitions 32t+g (matmul PSUM outputs
                # must start at a 0/32/64/96 base partition)
                rp = mpp.tile([100, N], F32, tag="rp")
                mp = [mpp.tile([128, N], F32, tag=f"mp{t}", name=f"mp{t}")
                      for t in range(NT)]
                for ct in range(NCT):
                    q, cr = divmod(ct, 4)
                    for t in range(NT):
                        nc.tensor.matmul(
                            mp[t][:],
                            tq[q][:, cr * FK + t * 128:cr * FK + (t + 1) * 128],
                            xq[q][:, cr * N:(cr + 1) * N],
                            start=(ct == 0),
                            stop=(ct == NCT - 1),
                        )
                for t in range(NT):
                    mt = pp.tile([128, N], BF16, tag=f"Mt{t}")
                    nc.vector.tensor_copy(mt[:], mp[t][:])
                    Mts.append(mt)
                    sq = pp.tile([128, N], F32, tag=f"sq{t}")
                    nc.vector.tensor_tensor(sq[:], mt[:], mt[:],
                                            mybir.AluOpType.mult)
                    nc.tensor.matmul(
                        rp[32 * t:32 * t + NT, :], Ofp[:], sq[:],
                        start=True, stop=True, tile_position=(0, 32 * t),
                    )
                    nc.vector.tensor_scalar(
                        rb[32 * t:32 * t + NT, :],
                        rp[32 * t:32 * t + NT, :],
                        -1.0 / 16.0,
                        None,
                        mybir.AluOpType.mult,
                    )
                    for g in range(NT):
                        f = NT * t + g
                        row = rb[32 * t + g:32 * t + g + 1, :]
                        nc.scalar.dma_start(
                            RFa[32 * g:32 * g + 1, f * N:(f + 1) * N], row)
                        nc.scalar.dma_start(
                            RFb[32 * g + 1:32 * g + 2, f * N:(f + 1) * N], row)

            # ---- phase 2: per-feature gram + fixup + relu-sum cert ----
            # bank cols [0:256) = P[i in B0, all j]; cols [256:384) =
            # P[i in B1, j in B1] (the (B1,B0) block is the transpose of
            # (B0,B1), already covered by h=0).
            gp_ctx = tc.tile_pool(name="gpsum", bufs=6,
                                  space=bass.MemorySpace.PSUM)
            gpp = gp_ctx.__enter__()
            for f in range(F_LOC):
                t, a = divmod(f, NT)
                bank = gpp.tile([128, BANK_W], F32, tag="gram")
                ms = Mts[t][32 * a:32 * a + 32, :]
                for h, (j0, w) in enumerate(((0, 2 * 128), (128, 128))):
                    seg = bank[:, 128 * (2 * h):128 * (2 * h) + w]
                    nc.tensor.matmul(
                        seg,
                        Mts[t][32 * a:32 * a + 32,
                               h * 128:h * 128 + 128],
                        Mts[t][32 * a:32 * a + 32, j0:j0 + w],
                        start=True,
                        stop=False,
                        tile_position=(32 * a, 0),
                    )
                    nc.tensor.matmul(
                        seg,
                        RFa[32 * a:32 * a + 2,
                            f * N + h * 128:f * N + h * 128 + 128],
                        RFb[32 * a:32 * a + 2, f * N + j0:f * N + j0 + w],
                        start=False,
                        stop=True,
                        tile_position=(32 * a, 0),
                    )
                if f % 3 != 2:
                    sa = scp.tile([128, BANK_W], BF16, tag="scrA")
                    nc.scalar.activation(
                        sa[:],
                        bank[:],
                        mybir.ActivationFunctionType.Relu,
                        bias=bth[:],
                        scale=1.0,
                        accum_out=certA[:, f:f + 1],
                    )
                else:
                    sd = scp.tile([128, BANK_W], BF16, tag="scrD")
                    nc.vector.tensor_scalar(
                        sd[:], bank[:], TH / 2, 0.0,
                        mybir.AluOpType.add, mybir.AluOpType.max,
                    )
                    nc.vector.tensor_reduce(
                        certD[:, f:f + 1], sd[:],
                        mybir.AxisListType.X, mybir.AluOpType.add,
                    )

            nc.sync.dma_start(certA_d[:], certA[:])
            nc.sync.dma_start(certD_d[:], certD[:])
            gp_ctx.__exit__(None, None, None)

    nc.compile()
    return nc


def _get_nc():
    if "nc" not in _CACHE:
        _CACHE["nc"] = _build()
    return _CACHE["nc"]


def _prep_inputs(x, T):
    x = np.asarray(x, dtype=np.float32)
    T = np.asarray(T, dtype=np.float32)
    xT = np.ascontiguousarray(x.T).astype(ml_dtypes.bfloat16)     # [2048, 256]
    xP = xT.reshape(NCT, 128, N).transpose(1, 0, 2)               # [128,16,256]
    Eig = np.full((1, F_LOC * N), 8.0, dtype=np.float16)
    Ofp = np.zeros((128, NT), dtype=np.float32)
    idx = np.arange(128)
    Ofp[idx, idx // KD] = 1.0
    shared = {"Eights": Eig, "Ofp": Ofp}
    for q in range(NQ):
        shared[f"xq{q}"] = np.ascontiguousarray(
            xP[:, 4 * q:4 * q + 4, :].reshape(128, 4 * N))
    in_maps = []
    for c in range(NCORES):
        f0 = c * F_LOC
        Tsl = T[:, f0:f0 + F_LOC, :].reshape(IN_F, FK).astype(ml_dtypes.bfloat16)
        TP = Tsl.reshape(NCT, 128, FK).transpose(1, 0, 2)         # [128,16,512]
        m = dict(shared)
        for q in range(NQ):
            m[f"tq{q}"] = np.ascontiguousarray(
                TP[:, 4 * q:4 * q + 4, :].reshape(128, 4 * FK))
        in_maps.append(m)
    return x, T, in_maps


def _exact_o_b(x, T):
    """Exact f32 o_b, matching the reference's underflow behavior."""
    M = (x @ T.reshape(IN_F, OUT_F * KD)).reshape(N, OUT_F, KD)
    o_b = np.zeros((N, OUT_F), dtype=np.float32)
    for i0 in range(0, N, 32):
        d = np.abs(M[i0:i0 + 32, None, :, :] - M[None, :, :, :]).sum(
            axis=3, dtype=np.float32
        )
        o_b += np.exp(-d.astype(np.float32)).sum(axis=0, dtype=np.float32)
    return o_b


def _run(x, T, trace=False):
    nc = _get_nc()
    x, T, in_maps = _prep_inputs(x, T)
    res = run_bass_kernel_spmd(nc, in_maps, core_ids=list(range(NCORES)), trace=trace)
    fsel = np.arange(F_LOC)
    acols = fsel[fsel % 3 != 2]
    dcols = fsel[fsel % 3 == 2]
    flagged = False
    for c in range(NCORES):
        certA = res.results[c]["certA"]                 # [128, F_LOC]
        certD = res.results[c]["certD"]
        if np.abs(certA[:, acols] - TH).max() > BAND:
            flagged = True
        if np.abs(certD[:, dcols] - TH).max() > BAND:
            flagged = True
    if flagged:
        o_b = _exact_o_b(x, T)
    else:
        o_b = np.ones((N, OUT_F), dtype=np.float32)
    out = np.concatenate([x, o_b], axis=1)
    return out, res


def kernel(x, T):
    out, _ = _run(x, T, trace=False)
    return out


# revision 18
# speedup vs baseline: 1.1121x; 1.1121x over previous
"""Trainium2 Bass kernel for MinibatchDiscrimination (screening formulation).

Reference computation:
    M = (x @ T.reshape(2048, 4096)).reshape(256, 128, 32)       # "matrices"
    norm1[i,j,f] = sum_k |M[i,f,k] - M[j,f,k]|                   (L1 over k)
    o_b[j,f]    = sum_i exp(-norm1[i,j,f])
    out         = concat([x, o_b], axis=1)                       # [256, 2176]

Key observation: in f32, exp(-z) is exactly 0.0 for z > 104 (the result
is below the smallest subnormal). For this problem M has std ~45, so
pairwise L1 norms are ~1600 and every off-diagonal exp underflows to an
exact 0 in the f32 reference; o_b is exactly ones + corrections from
any "close" pair. The kernel therefore SCREENS: it lower-bounds every
off-diagonal L1 norm with the pairwise L2 norm (norm1 >= norm2), which
is computable as per-feature gram matrices on the tensor engine at ~50x
the throughput of the elementwise L1 pass. Rows whose bound cannot
certify underflow are recomputed exactly on the host (f32, matching the
reference); for generic inputs nothing is flagged.

Sharding: OUT_FEATURES (128) split across 8 cores (16 features each),
no collectives, no duplicated matmul work.

Device computation per core:
  M^T tiles [128 fk', 256 i] (fk' = f*32+k, bf16) via PE;
  r[f,i] = sum_k M[i,f,k]^2 via DVE squares + fp32 ones matmul;
  per feature f: PSUM bank holds
      P[i,j] = G_ij - r_i/2 - r_j/2  (= -norm2^2/2)
  from a 32-contraction gram matmul plus a rank-2 fp16 fixup matmul
  ((-r/16) x 8 + 8 x (-r/16)); the h=1 half computes only the (B1,B1)
  block (the (B1,B0) block is the transpose of (B0,B1), already
  covered), so the bank is [128, 384].

Certificate (relu-sum, no diagonal mask):
    cert[p,f] = sum_j relu(P + TH/2) over the bank row p
  Healthy: every off-diagonal P <= -TH/2 so only the two diagonal
  entries contribute: cert = 2*relu(TH/2 + delta) = TH + O(delta),
  where |delta| <= ~128 (fp16 r/16 rounding). Host accepts iff
  |cert - TH| <= BAND.
  Soundness (TH=17800, BAND=400): acceptance implies the total hidden
  relu leak is <= BAND + 2*|delta| + rounding ~= 700, so every
  off-diag P <= -TH/2 + 700 => device-norm2^2 >= TH - 1400 = 16400 =>
  bf16-L2 >= 128 => true f32 L2 >= 116 (bf16-M error <= ~1/entry,
  ||dd||_2 <= 12) => true L1 >= 116 > 104 => reference entry is
  exactly 0. Actual min off-diag norm2^2 is ~18666 > TH, so no leaks
  and no flags on generic inputs; any flag falls back to an exact
  host recompute (still correct, just slow).
  ACT banks use activation(Relu, bias=TH/2, accum_out); DVE banks use
  tensor_scalar(add TH/2, max 0) + tensor_reduce(add). Both healthy
  values are TH + O(delta) (DVE adds bf16 rounding of the two ~TH/2
  diagonal relus, ~ +-64, inside BAND).
"""

import sys

if "/opt/trn_rl_repo" not in sys.path:
    sys.path.insert(0, "/opt/trn_rl_repo")

import ml_dtypes
import numpy as np

import concourse.bacc as bacc
import concourse.bass as bass
import concourse.mybir as mybir
import concourse.tile as tile
from concourse.bass_utils import run_bass_kernel_spmd

N = 256
IN_F = 2048
OUT_F = 128
KD = 32
NCORES = 8
F_LOC = OUT_F // NCORES        # 16 features per core
FK = F_LOC * KD                # 512 (f-major: fk = f*32 + k)
NT = FK // 128                 # 4 fk tiles of 128 partitions (4 features each)
NCT = IN_F // 128              # 16 contraction tiles
NQ = 4                         # input DMA chunks (4 ct each)

TH = 17800.0                   # norm2^2 certification threshold
BAND = 400.0                   # |cert - TH| acceptance band
BANK_W = 3 * 128               # 256 (h0 full) + 128 (h1 B1xB1 block)

F32 = mybir.dt.float32
BF16 = mybir.dt.bfloat16
FP16 = mybir.dt.float16

_CACHE = {}


def _build():
    nc = bacc.Bacc()
    xq_d = [nc.dram_tensor(f"xq{q}", [128, 4 * N], BF16, kind="ExternalInput")
            for q in range(NQ)]
    tq_d = [nc.dram_tensor(f"tq{q}", [128, 4 * FK], BF16, kind="ExternalInput")
            for q in range(NQ)]
    Eig_d = nc.dram_tensor("Eights", [1, F_LOC * N], FP16, kind="ExternalInput")
    Ofp_d = nc.dram_tensor("Ofp", [128, NT], F32, kind="ExternalInput")
    certA_d = nc.dram_tensor("certA", [128, F_LOC], F32, kind="ExternalOutput")
    certD_d = nc.dram_tensor("certD", [128, F_LOC], F32, kind="ExternalOutput")

    with tile.TileContext(nc) as tc:
        with (
            tc.tile_pool(name="persist", bufs=1) as pp,
            tc.tile_pool(name="scr", bufs=4) as scp,
        ):
            # ---- input DMAs: packed fat rows, M inputs first ----
            xq = []
            tq = []
            for q in range(NQ):
                xs = pp.tile([128, 4 * N], BF16, tag=f"xq{q}")
                nc.sync.dma_start(xs[:], xq_d[q][:])
                xq.append(xs)
                ts = pp.tile([128, 4 * FK], BF16, tag=f"tq{q}")
                nc.sync.dma_start(ts[:], tq_d[q][:])
                tq.append(ts)

            # fixup operands, co-located with the gram's PE row tile
            # (partitions 32a, 32a+1): different PE row tiles must not
            # accumulate into the same PSUM bank.
            # RFa rows: 32a = -r/16 (stationary), 32a+1 = 8.0
            # RFb rows: 32a = 8.0 (moving),       32a+1 = -r/16
            RFa = pp.tile([98, F_LOC * N], FP16, tag="RFa")
            RFb = pp.tile([98, F_LOC * N], FP16, tag="RFb")
            for a in range(NT):
                nc.sync.dma_start(RFa[32 * a + 1:32 * a + 2, :], Eig_d[:])
                nc.sync.dma_start(RFb[32 * a:32 * a + 1, :], Eig_d[:])
            Ofp = pp.tile([128, NT], F32, tag="Ofp")
            nc.sync.dma_start(Ofp[:], Ofp_d[:])

            # separate per-engine cert tiles: a shared tile would make
            # Tile serialize the ACT and DVE drains against each other
            certA = pp.tile([128, F_LOC], F32, tag="certA")
            nc.vector.memset(certA[:], 0.0)
            certD = pp.tile([128, F_LOC], F32, tag="certD")
            nc.vector.memset(certD[:], 0.0)
            bth = pp.tile([128, 1], F32, tag="bth")
            nc.vector.memset(bth[:], TH / 2)
            rb = pp.tile([100, N], FP16, tag="rb")

            # ---- phase 1: M^T tiles [128 fk', 256 i], ct-outer so the
            # tensor engine starts after the first input chunk; the PSUM
            # pool closes before phase 2 so the gram pool gets 6 banks ----
            Mts = []
            with tc.tile_pool(name="mpsum", bufs=1,
                              space=bass.MemorySpace.PSUM) as mpp:
                # r rows live at partitions 32t+g (matmul PSUM outputs
                # must start at a 0/32/64/96 base partition)
                rp = mpp.tile([100, N], F32, tag="rp")
                mp = [mpp.tile([128, N], F32, tag=f"mp{t}", name=f"mp{t}")
                      for t in range(NT)]
                for ct in range(NCT):
                    q, cr = divmod(ct, 4)
                    for t in range(NT):
                        nc.tensor.matmul(
                            mp[t][:],
                            tq[q][:, cr * FK + t * 128:cr * FK + (t + 1) * 128],
                            xq[q][:, cr * N:(cr + 1) * N],
                            start=(ct == 0),
                            stop=(ct == NCT - 1),
                        )
                for t in range(NT):
                    mt = pp.tile([128, N], BF16, tag=f"Mt{t}")
                    nc.vector.tensor_copy(mt[:], mp[t][:])
                    Mts.append(mt)
                    sq = pp.tile([128, N], F32, tag=f"sq{t}")
                    nc.vector.tensor_tensor(sq[:], mt[:], mt[:],
                                            mybir.AluOpType.mult)
                    nc.tensor.matmul(
                        rp[32 * t:32 * t + NT, :], Ofp[:], sq[:],
                        start=True, stop=True, tile_position=(0, 32 * t),
                    )
                    nc.vector.tensor_scalar(
                        rb[32 * t:32 * t + NT, :],
                        rp[32 * t:32 * t + NT, :],
                        -1.0 / 16.0,
                        None,
                        mybir.AluOpType.mult,
                    )
                    for g in range(NT):
                        f = NT * t + g
                        row = rb[32 * t + g:32 * t + g + 1, :]
                        nc.sync.dma_start(
                            RFa[32 * g:32 * g + 1, f * N:(f + 1) * N], row)
                        nc.sync.dma_start(
                            RFb[32 * g + 1:32 * g + 2, f * N:(f + 1) * N], row)

            # ---- phase 2: per-feature gram + fixup + relu-sum cert ----
            # bank cols [0:256) = P[i in B0, all j]; cols [256:384) =
            # P[i in B1, j in B1] (the (B1,B0) block is the transpose of
            # (B0,B1), already covered by h=0).
            gp_ctx = tc.tile_pool(name="gpsum", bufs=6,
                                  space=bass.MemorySpace.PSUM)
            gpp = gp_ctx.__enter__()
            for f in range(F_LOC):
                t, a = divmod(f, NT)
                bank = gpp.tile([128, BANK_W], F32, tag="gram")
                ms = Mts[t][32 * a:32 * a + 32, :]
                for h, (j0, w) in enumerate(((0, 2 * 128), (128, 128))):
                    seg = bank[:, 128 * (2 * h):128 * (2 * h) + w]
                    nc.tensor.matmul(
                        seg,
                        Mts[t][32 * a:32 * a + 32,
                               h * 128:h * 128 + 128],
                        Mts[t][32 * a:32 * a + 32, j0:j0 + w],
                        start=True,
                        stop=False,
                        tile_position=(32 * a, 0),
                    )
                    nc.tensor.matmul(
                        seg,
                        RFa[32 * a:32 * a + 2,
                            f * N + h * 128:f * N + h * 128 + 128],
                        RFb[32 * a:32 * a + 2, f * N + j0:f * N + j0 + w],
                        start=False,
                        stop=True,
                        tile_position=(32 * a, 0),
                    )
                if f % 3 != 2:
                    sa = scp.tile([128, BANK_W], BF16, tag="scrA")
                    nc.scalar.activation(
                        sa[:],
                        bank[:],
                        mybir.ActivationFunctionType.Relu,
                        bias=bth[:],
                        scale=1.0,
                        accum_out=certA[:, f:f + 1],
                    )
                else:
                    sd = scp.tile([128, BANK_W], BF16, tag="scrD")
                    nc.vector.tensor_scalar(
                        sd[:], bank[:], TH / 2, 0.0,
                        mybir.AluOpType.add, mybir.AluOpType.max,
                    )
                    nc.vector.tensor_reduce(
                        certD[:, f:f + 1], sd[:],
                        mybir.AxisListType.X, mybir.AluOpType.add,
                    )

            nc.sync.dma_start(certA_d[:], certA[:])
            nc.sync.dma_start(certD_d[:], certD[:])
            gp_ctx.__exit__(None, None, None)

    nc.compile()
    return nc


def _get_nc():
    if "nc" not in _CACHE:
        _CACHE["nc"] = _build()
    return _CACHE["nc"]


def _prep_inputs(x, T):
    x = np.asarray(x, dtype=np.float32)
    T = np.asarray(T, dtype=np.float32)
    xT = np.ascontiguousarray(x.T).astype(ml_dtypes.bfloat16)     # [2048, 256]
    xP = xT.reshape(NCT, 128, N).transpose(1, 0, 2)               # [128,16,256]
    Eig = np.full((1, F_LOC * N), 8.0, dtype=np.float16)
    Ofp = np.zeros((128, NT), dtype=np.float32)
    idx = np.arange(128)
    Ofp[idx, idx // KD] = 1.0
    shared = {"Eights": Eig, "Ofp": Ofp}
    for q in range(NQ):
        shared[f"xq{q}"] = np.ascontiguousarray(
            xP[:, 4 * q:4 * q + 4, :].reshape(128, 4 * N))
    in_maps = []
    for c in range(NCORES):
        f0 = c * F_LOC
        Tsl = T[:, f0:f0 + F_LOC, :].reshape(IN_F, FK).astype(ml_dtypes.bfloat16)
        TP = Tsl.reshape(NCT, 128, FK).transpose(1, 0, 2)         # [128,16,512]
        m = dict(shared)
        for q in range(NQ):
            m[f"tq{q}"] = np.ascontiguousarray(
                TP[:, 4 * q:4 * q + 4, :].reshape(128, 4 * FK))
        in_maps.append(m)
    return x, T, in_maps


def _exact_o_b(x, T):
    """Exact f32 o_b, matching the reference's underflow behavior."""
    M = (x @ T.reshape(IN_F, OUT_F * KD)).reshape(N, OUT_F, KD)
    o_b = np.zeros((N, OUT_F), dtype=np.float32)
    for i0 in range(0, N, 32):
        d = np.abs(M[i0:i0 + 32, None, :, :] - M[None, :, :, :]).sum(
            axis=3, dtype=np.float32
        )
        o_b += np.exp(-d.astype(np.float32)).sum(axis=0, dtype=np.float32)
    return o_b


def _run(x, T, trace=False):
    nc = _get_nc()
    x, T, in_maps = _prep_inputs(x, T)
    res = run_bass_kernel_spmd(nc, in_maps, core_ids=list(range(NCORES)), trace=trace)
    fsel = np.arange(F_LOC)
    acols = fsel[fsel % 3 != 2]
    dcols = fsel[fsel % 3 == 2]
    flagged = False
    for c in range(NCORES):
        certA = res.results[c]["certA"]                 # [128, F_LOC]
        certD = res.results[c]["certD"]
        if np.abs(certA[:, acols] - TH).max() > BAND:
            flagged = True
        if np.abs(certD[:, dcols] - TH).max() > BAND:
            flagged = True
    if flagged:
        o_b = _exact_o_b(x, T)
    else:
        o_b = np.ones((N, OUT_F), dtype=np.float32)
    out = np.concatenate([x, o_b], axis=1)
    return out, res


def kernel(x, T):
    out, _ = _run(x, T, trace=False)
    return out


# revision 20
# speedup vs baseline: 1.1779x; 1.0592x over previous
"""Trainium2 Bass kernel for MinibatchDiscrimination (screening formulation).

Reference computation:
    M = (x @ T.reshape(2048, 4096)).reshape(256, 128, 32)       # "matrices"
    norm1[i,j,f] = sum_k |M[i,f,k] - M[j,f,k]|                   (L1 over k)
    o_b[j,f]    = sum_i exp(-norm1[i,j,f])
    out         = concat([x, o_b], axis=1)                       # [256, 2176]

Key observation: in f32, exp(-z) is exactly 0.0 for z > 104 (the result
is below the smallest subnormal). For this problem M has std ~45, so
pairwise L1 norms are ~1600 and every off-diagonal exp underflows to an
exact 0 in the f32 reference; o_b is exactly ones + corrections from
any "close" pair. The kernel therefore SCREENS: it lower-bounds every
off-diagonal L1 norm with the pairwise L2 norm (norm1 >= norm2), which
is computable as per-feature gram matrices on the tensor engine at ~50x
the throughput of the elementwise L1 pass. Rows whose bound cannot
certify underflow are recomputed exactly on the host (f32, matching the
reference); for generic inputs nothing is flagged.

Sharding: OUT_FEATURES (128) split across 8 cores (16 features each),
no collectives, no duplicated matmul work.

Device computation per core:
  M^T tiles [128 fk', 256 i] (fk' = f*32+k, bf16) via PE;
  r[f,i] = sum_k M[i,f,k]^2 via DVE squares + fp32 ones matmul;
  per feature f: PSUM bank holds
      P[i,j] = G_ij - r_i/2 - r_j/2  (= -norm2^2/2)
  from a 32-contraction gram matmul plus a rank-2 fp16 fixup matmul
  ((-r/16) x 8 + 8 x (-r/16)); the h=1 half computes only the (B1,B1)
  block (the (B1,B0) block is the transpose of (B0,B1), already
  covered), so the bank is [128, 384].

Certificate (relu-sum, no diagonal mask):
    cert[p,f] = sum_j relu(P + TH/2) over the bank row p
  Healthy: every off-diagonal P <= -TH/2 so only the two diagonal
  entries contribute: cert = 2*relu(TH/2 + delta) = TH + O(delta),
  where |delta| <= ~128 (fp16 r/16 rounding). Host accepts iff
  |cert - TH| <= BAND.
  Soundness (TH=17800, BAND=400): acceptance implies the total hidden
  relu leak is <= BAND + 2*|delta| + rounding ~= 700, so every
  off-diag P <= -TH/2 + 700 => device-norm2^2 >= TH - 1400 = 16400 =>
  bf16-L2 >= 128 => true f32 L2 >= 116 (bf16-M error <= ~1/entry,
  ||dd||_2 <= 12) => true L1 >= 116 > 104 => reference entry is
  exactly 0. Actual min off-diag norm2^2 is ~18666 > TH, so no leaks
  and no flags on generic inputs; any flag falls back to an exact
  host recompute (still correct, just slow).
  ACT banks use activation(Relu, bias=TH/2, accum_out); DVE banks use
  tensor_scalar(add TH/2, max 0) + tensor_reduce(add). Both healthy
  values are TH + O(delta) (DVE adds bf16 rounding of the two ~TH/2
  diagonal relus, ~ +-64, inside BAND).
"""

import sys

if "/opt/trn_rl_repo" not in sys.path:
    sys.path.insert(0, "/opt/trn_rl_repo")

import ml_dtypes
import numpy as np

import concourse.bacc as bacc
import concourse.bass as bass
import concourse.mybir as mybir
import concourse.tile as tile
from concourse.bass_utils import run_bass_kernel_spmd

N = 256
IN_F = 2048
OUT_F = 128
KD = 32
NCORES = 8
F_LOC = OUT_F // NCORES        # 16 features per core
FK = F_LOC * KD                # 512 (f-major: fk = f*32 + k)
NT = FK // 128                 # 4 fk tiles of 128 partitions (4 features each)
NCT = IN_F // 128              # 16 contraction tiles
NQ = 4                         # input DMA chunks (4 ct each)

TH = 17800.0                   # norm2^2 certification threshold
BAND = 400.0                   # |cert - TH| acceptance band
BANK_W = 3 * 128               # 256 (h0 full) + 128 (h1 B1xB1 block)

F32 = mybir.dt.float32
BF16 = mybir.dt.bfloat16
FP16 = mybir.dt.float16

_CACHE = {}


def _build():
    nc = bacc.Bacc()
    xq_d = [nc.dram_tensor(f"xq{q}", [128, 4 * N], BF16, kind="ExternalInput")
            for q in range(NQ)]
    tq_d = [nc.dram_tensor(f"tq{q}", [128, 4 * FK], BF16, kind="ExternalInput")
            for q in range(NQ)]
    Eig_d = nc.dram_tensor("Eights", [1, NT * N], FP16, kind="ExternalInput")
    Ofp_d = nc.dram_tensor("Ofp", [128, NT], F32, kind="ExternalInput")
    certA_d = nc.dram_tensor("certA", [128, F_LOC], F32, kind="ExternalOutput")
    certD_d = nc.dram_tensor("certD", [128, F_LOC], F32, kind="ExternalOutput")

    with tile.TileContext(nc) as tc:
        with (
            tc.tile_pool(name="persist", bufs=1) as pp,
            tc.tile_pool(name="scr", bufs=4) as scp,
        ):
            # ---- input DMAs: packed fat rows, M inputs first ----
            xq = []
            tq = []
            for q in range(NQ):
                xs = pp.tile([128, 4 * N], BF16, tag=f"xq{q}")
                nc.sync.dma_start(xs[:], xq_d[q][:])
                xq.append(xs)
                ts = pp.tile([128, 4 * FK], BF16, tag=f"tq{q}")
                nc.sync.dma_start(ts[:], tq_d[q][:])
                tq.append(ts)

            # fixup operands, co-located with the gram's PE row tile
            # (partitions 32a, 32a+1): different PE row tiles must not
            # accumulate into the same PSUM bank.
            # RFa rows: 32a = -r/16 (stationary), 32a+1 = 8.0
            # RFb rows: 32a = 8.0 (moving),       32a+1 = -r/16
            # t-major columns: feature f = 4t+g lives at partition 32g,
            # cols [t*N, (t+1)*N)
            RFa = pp.tile([98, NT * N], FP16, tag="RFa")
            RFb = pp.tile([98, NT * N], FP16, tag="RFb")
            for a in range(NT):
                nc.sync.dma_start(RFa[32 * a + 1:32 * a + 2, :], Eig_d[:])
                nc.sync.dma_start(RFb[32 * a:32 * a + 1, :], Eig_d[:])
            Ofp = pp.tile([128, NT], F32, tag="Ofp")
            nc.sync.dma_start(Ofp[:], Ofp_d[:])

            # separate per-engine cert tiles: a shared tile would make
            # Tile serialize the ACT and DVE drains against each other
            certA = pp.tile([128, F_LOC], F32, tag="certA")
            nc.vector.memset(certA[:], 0.0)
            certD = pp.tile([128, F_LOC], F32, tag="certD")
            nc.vector.memset(certD[:], 0.0)
            bth = pp.tile([128, 1], F32, tag="bth")
            nc.vector.memset(bth[:], TH / 2)
            rb = pp.tile([100, N], FP16, tag="rb")

            # ---- phase 1: M^T tiles [128 fk', 256 i], ct-outer so the
            # tensor engine starts after the first input chunk; the PSUM
            # pool closes before phase 2 so the gram pool gets 6 banks ----
            Mts = []
            with tc.tile_pool(name="mpsum", bufs=1,
                              space=bass.MemorySpace.PSUM) as mpp:
                # r rows live at partitions 32t+g (matmul PSUM outputs
                # must start at a 0/32/64/96 base partition)
                rp = mpp.tile([100, N], F32, tag="rp")
                mp = [mpp.tile([128, N], F32, tag=f"mp{t}", name=f"mp{t}")
                      for t in range(NT)]
                for ct in range(NCT):
                    q, cr = divmod(ct, 4)
                    for t in range(NT):
                        nc.tensor.matmul(
                            mp[t][:],
                            tq[q][:, cr * FK + t * 128:cr * FK + (t + 1) * 128],
                            xq[q][:, cr * N:(cr + 1) * N],
                            start=(ct == 0),
                            stop=(ct == NCT - 1),
                        )
                for t in range(NT):
                    mt = pp.tile([128, N], BF16, tag=f"Mt{t}")
                    nc.vector.tensor_copy(mt[:], mp[t][:])
                    Mts.append(mt)
                    sq = pp.tile([128, N], F32, tag=f"sq{t}")
                    nc.vector.tensor_tensor(sq[:], mt[:], mt[:],
                                            mybir.AluOpType.mult)
                    nc.tensor.matmul(
                        rp[32 * t:32 * t + NT, :], Ofp[:], sq[:],
                        start=True, stop=True, tile_position=(0, 32 * t),
                    )
                    nc.vector.tensor_scalar(
                        rb[32 * t:32 * t + NT, :],
                        rp[32 * t:32 * t + NT, :],
                        -1.0 / 16.0,
                        None,
                        mybir.AluOpType.mult,
                    )
                    # scatters issued from the otherwise-idle gpsimd
                    # engine: SP-issued tiny DMAs paced the whole screen
                    # phase (~600ns per SP instruction)
                    for g in range(NT):
                        row = rb[32 * t + g:32 * t + g + 1, :]
                        nc.gpsimd.dma_start(
                            RFa[32 * g:32 * g + 1, t * N:(t + 1) * N], row)
                        nc.gpsimd.dma_start(
                            RFb[32 * g + 1:32 * g + 2, t * N:(t + 1) * N], row)

            # ---- phase 2: per-feature gram + fixup + relu-sum cert ----
            # bank cols [0:256) = P[i in B0, all j]; cols [256:384) =
            # P[i in B1, j in B1] (the (B1,B0) block is the transpose of
            # (B0,B1), already covered by h=0).
            gp_ctx = tc.tile_pool(name="gpsum", bufs=6,
                                  space=bass.MemorySpace.PSUM)
            gpp = gp_ctx.__enter__()
            for f in range(F_LOC):
                t, a = divmod(f, NT)
                bank = gpp.tile([128, BANK_W], F32, tag="gram")
                ms = Mts[t][32 * a:32 * a + 32, :]
                for h, (j0, w) in enumerate(((0, 2 * 128), (128, 128))):
                    seg = bank[:, 128 * (2 * h):128 * (2 * h) + w]
                    nc.tensor.matmul(
                        seg,
                        Mts[t][32 * a:32 * a + 32,
                               h * 128:h * 128 + 128],
                        Mts[t][32 * a:32 * a + 32, j0:j0 + w],
                        start=True,
                        stop=False,
                        tile_position=(32 * a, 0),
                    )
                    nc.tensor.matmul(
                        seg,
                        RFa[32 * a:32 * a + 2,
                            t * N + h * 128:t * N + h * 128 + 128],
                        RFb[32 * a:32 * a + 2, t * N + j0:t * N + j0 + w],
                        start=False,
                        stop=True,
                        tile_position=(32 * a, 0),
                    )
                if f % 3 != 2:
                    sa = scp.tile([128, BANK_W], BF16, tag="scrA")
                    nc.scalar.activation(
                        sa[:],
                        bank[:],
                        mybir.ActivationFunctionType.Relu,
                        bias=bth[:],
                        scale=1.0,
                        accum_out=certA[:, f:f + 1],
                    )
                else:
                    sd = scp.tile([128, BANK_W], BF16, tag="scrD")
                    nc.vector.tensor_scalar(
                        sd[:], bank[:], TH / 2, 0.0,
                        mybir.AluOpType.add, mybir.AluOpType.max,
                    )
                    nc.vector.tensor_reduce(
                        certD[:, f:f + 1], sd[:],
                        mybir.AxisListType.X, mybir.AluOpType.add,
                    )

            nc.sync.dma_start(certA_d[:], certA[:])
            nc.sync.dma_start(certD_d[:], certD[:])
            gp_ctx.__exit__(None, None, None)

    nc.compile()
    return nc


def _get_nc():
    if "nc" not in _CACHE:
        _CACHE["nc"] = _build()
    return _CACHE["nc"]


def _prep_inputs(x, T):
    x = np.asarray(x, dtype=np.float32)
    T = np.asarray(T, dtype=np.float32)
    xT = np.ascontiguousarray(x.T).astype(ml_dtypes.bfloat16)     # [2048, 256]
    xP = xT.reshape(NCT, 128, N).transpose(1, 0, 2)               # [128,16,256]
    Eig = np.full((1, NT * N), 8.0, dtype=np.float16)
    Ofp = np.zeros((128, NT), dtype=np.float32)
    idx = np.arange(128)
    Ofp[idx, idx // KD] = 1.0
    shared = {"Eights": Eig, "Ofp": Ofp}
    for q in range(NQ):
        shared[f"xq{q}"] = np.ascontiguousarray(
            xP[:, 4 * q:4 * q + 4, :].reshape(128, 4 * N))
    in_maps = []
    for c in range(NCORES):
        f0 = c * F_LOC
        Tsl = T[:, f0:f0 + F_LOC, :].reshape(IN_F, FK).astype(ml_dtypes.bfloat16)
        TP = Tsl.reshape(NCT, 128, FK).transpose(1, 0, 2)         # [128,16,512]
        m = dict(shared)
        for q in range(NQ):
            m[f"tq{q}"] = np.ascontiguousarray(
                TP[:, 4 * q:4 * q + 4, :].reshape(128, 4 * FK))
        in_maps.append(m)
    return x, T, in_maps


def _exact_o_b(x, T):
    """Exact f32 o_b, matching the reference's underflow behavior."""
    M = (x @ T.reshape(IN_F, OUT_F * KD)).reshape(N, OUT_F, KD)
    o_b = np.zeros((N, OUT_F), dtype=np.float32)
    for i0 in range(0, N, 32):
        d = np.abs(M[i0:i0 + 32, None, :, :] - M[None, :, :, :]).sum(
            axis=3, dtype=np.float32
        )
        o_b += np.exp(-d.astype(np.float32)).sum(axis=0, dtype=np.float32)
    return o_b


def _run(x, T, trace=False):
    nc = _get_nc()
    x, T, in_maps = _prep_inputs(x, T)
    res = run_bass_kernel_spmd(nc, in_maps, core_ids=list(range(NCORES)), trace=trace)
    fsel = np.arange(F_LOC)
    acols = fsel[fsel % 3 != 2]
    dcols = fsel[fsel % 3 == 2]
    flagged = False
    for c in range(NCORES):
        certA = res.results[c]["certA"]                 # [128, F_LOC]
        certD = res.results[c]["certD"]
        if np.abs(certA[:, acols] - TH).max() > BAND:
            flagged = True
        if np.abs(certD[:, dcols] - TH).max() > BAND:
            flagged = True
    if flagged:
        o_b = _exact_o_b(x, T)
    else:
        o_b = np.ones((N, OUT_F), dtype=np.float32)
    out = np.concatenate([x, o_b], axis=1)
    return out, res


def kernel(x, T):
    out, _ = _run(x, T, trace=False)
    return out


# revision 21
# speedup vs baseline: 1.6289x; 1.3829x over previous
"""Trainium2 Bass kernel for MinibatchDiscrimination (screening formulation).

Reference computation:
    M = (x @ T.reshape(2048, 4096)).reshape(256, 128, 32)       # "matrices"
    norm1[i,j,f] = sum_k |M[i,f,k] - M[j,f,k]|                   (L1 over k)
    o_b[j,f]    = sum_i exp(-norm1[i,j,f])
    out         = concat([x, o_b], axis=1)                       # [256, 2176]

Key observation: in f32, exp(-z) is exactly 0.0 for z > 104 (the result
is below the smallest subnormal). For this problem M has std ~45, so
pairwise L1 norms are ~1600 and every off-diagonal exp underflows to an
exact 0 in the f32 reference; o_b is exactly ones + corrections from
any "close" pair. The kernel therefore SCREENS: it lower-bounds every
off-diagonal L1 norm with the pairwise L2 norm (norm1 >= norm2), which
is computable as per-feature gram matrices on the tensor engine at ~50x
the throughput of the elementwise L1 pass. Rows whose bound cannot
certify underflow are recomputed exactly on the host (f32, matching the
reference); for generic inputs nothing is flagged.

Sharding: OUT_FEATURES (128) split across 8 cores (16 features each),
no collectives, no duplicated matmul work.

Device computation per core:
  M^T tiles [128 fk', 256 i] (fk' = f*32+k, bf16) via PE;
  r[f,i] = sum_k M[i,f,k]^2 via DVE squares + fp32 ones matmul;
  per feature f: PSUM bank holds
      P[i,j] = G_ij - r_i/2 - r_j/2  (= -norm2^2/2)
  from a 32-contraction gram matmul plus a rank-2 fp16 fixup matmul
  ((-r/16) x 8 + 8 x (-r/16)); the h=1 half computes only the (B1,B1)
  block (the (B1,B0) block is the transpose of (B0,B1), already
  covered), so the bank is [128, 384].

Certificate (relu-sum, no diagonal mask):
    cert[p,f] = sum_j relu(P + TH/2) over the bank row p
  Healthy: every off-diagonal P <= -TH/2 so only the two diagonal
  entries contribute: cert = 2*relu(TH/2 + delta) = TH + O(delta),
  where |delta| <= ~128 (fp16 r/16 rounding). Host accepts iff
  |cert - TH| <= BAND.
  Soundness (TH=17800, BAND=400): acceptance implies the total hidden
  relu leak is <= BAND + 2*|delta| + rounding ~= 700, so every
  off-diag P <= -TH/2 + 700 => device-norm2^2 >= TH - 1400 = 16400 =>
  bf16-L2 >= 128 => true f32 L2 >= 116 (bf16-M error <= ~1/entry,
  ||dd||_2 <= 12) => true L1 >= 116 > 104 => reference entry is
  exactly 0. Actual min off-diag norm2^2 is ~18666 > TH, so no leaks
  and no flags on generic inputs; any flag falls back to an exact
  host recompute (still correct, just slow).
  ACT banks use activation(Relu, bias=TH/2, accum_out); DVE banks use
  tensor_scalar(add TH/2, max 0) + tensor_reduce(add). Both healthy
  values are TH + O(delta) (DVE adds bf16 rounding of the two ~TH/2
  diagonal relus, ~ +-64, inside BAND).
"""

import sys

if "/opt/trn_rl_repo" not in sys.path:
    sys.path.insert(0, "/opt/trn_rl_repo")

import ml_dtypes
import numpy as np

import concourse.bacc as bacc
import concourse.bass as bass
import concourse.mybir as mybir
import concourse.tile as tile
from concourse.bass_utils import run_bass_kernel_spmd

N = 256
IN_F = 2048
OUT_F = 128
KD = 32
NCORES = 8
F_LOC = OUT_F // NCORES        # 16 features per core
FK = F_LOC * KD                # 512 (f-major: fk = f*32 + k)
NT = FK // 128                 # 4 fk tiles of 128 partitions (4 features each)
NCT = IN_F // 128              # 16 contraction tiles
NQ = 4                         # input DMA chunks (4 ct each)

TH = 17800.0                   # norm2^2 certification threshold
BAND = 700.0                   # |cert - TH| acceptance band
BANK_W = 3 * 128               # 256 (h0 full) + 128 (h1 B1xB1 block)

F32 = mybir.dt.float32
BF16 = mybir.dt.bfloat16
FP16 = mybir.dt.float16

_CACHE = {}


def _build():
    nc = bacc.Bacc()
    xq_d = [nc.dram_tensor(f"xq{q}", [128, 4 * N], BF16, kind="ExternalInput")
            for q in range(NQ)]
    tq_d = [nc.dram_tensor(f"tq{q}", [128, 4 * FK], BF16, kind="ExternalInput")
            for q in range(NQ)]
    Ofp_d = nc.dram_tensor("Ofp", [128, NT], BF16, kind="ExternalInput")
    certA_d = nc.dram_tensor("certA", [128, F_LOC], F32, kind="ExternalOutput")
    certD_d = nc.dram_tensor("certD", [128, F_LOC], F32, kind="ExternalOutput")

    with tile.TileContext(nc) as tc:
        with (
            tc.tile_pool(name="persist", bufs=1) as pp,
            tc.tile_pool(name="scr", bufs=4) as scp,
        ):
            # ---- input DMAs: packed fat rows, M inputs first ----
            xq = []
            tq = []
            for q in range(NQ):
                xs = pp.tile([128, 4 * N], BF16, tag=f"xq{q}")
                nc.sync.dma_start(xs[:], xq_d[q][:])
                xq.append(xs)
                ts = pp.tile([128, 4 * FK], BF16, tag=f"tq{q}")
                nc.sync.dma_start(ts[:], tq_d[q][:])
                tq.append(ts)

            # fixup operands: RFr holds -r/16 (fp16) at partition 32a,
            # cols [t*N, (t+1)*N) for feature f = 4t+a, written by 16
            # per-feature rank-1 r-matmuls + one DVE cast (no DMAs: tiny
            # SP/gpsimd-issued scatter DMAs paced the screen phase).
            # E8 is a constant 8.0 tile; fixup adds via two rank-1
            # matmuls (-r/16 x 8 + 8 x -r/16) in the gram's row tile.
            Ofp = pp.tile([128, NT], BF16, tag="Ofp")
            nc.sync.dma_start(Ofp[:], Ofp_d[:])
            RFr = pp.tile([97, NT * N], FP16, tag="RFr")
            E8 = pp.tile([97, N], FP16, tag="E8")
            nc.vector.memset(E8[:], 8.0)

            # separate per-engine cert tiles: a shared tile would make
            # Tile serialize the ACT and DVE drains against each other
            certA = pp.tile([128, F_LOC], F32, tag="certA")
            nc.vector.memset(certA[:], 0.0)
            certD = pp.tile([128, F_LOC], F32, tag="certD")
            nc.vector.memset(certD[:], 0.0)
            bth = pp.tile([128, 1], F32, tag="bth")
            nc.vector.memset(bth[:], TH / 2)

            # ---- phase 1: M^T tiles [128 fk', 256 i], ct-outer so the
            # tensor engine starts after the first input chunk; the PSUM
            # pool closes before phase 2 so the gram pool gets 6 banks ----
            Mts = []
            with tc.tile_pool(name="mpsum", bufs=1,
                              space=bass.MemorySpace.PSUM) as mpp:
                # per-feature r rows at partitions 32a, cols t*N
                rp = mpp.tile([97, NT * N], F32, tag="rp")
                nc.vector.memset(rp[:], 0.0)
                mp = [mpp.tile([128, N], F32, tag=f"mp{t}", name=f"mp{t}")
                      for t in range(NT)]
                for ct in range(NCT):
                    q, cr = divmod(ct, 4)
                    for t in range(NT):
                        nc.tensor.matmul(
                            mp[t][:],
                            tq[q][:, cr * FK + t * 128:cr * FK + (t + 1) * 128],
                            xq[q][:, cr * N:(cr + 1) * N],
                            start=(ct == 0),
                            stop=(ct == NCT - 1),
                        )
                for t in range(NT):
                    mt = pp.tile([128, N], BF16, tag=f"Mt{t}")
                    nc.vector.tensor_copy(mt[:], mp[t][:])
                    Mts.append(mt)
                    sq = pp.tile([128, N], BF16, tag=f"sq{t}")
                    nc.vector.tensor_tensor(sq[:], mt[:], mt[:],
                                            mybir.AluOpType.mult)
                    for g in range(NT):
                        nc.tensor.matmul(
                            rp[32 * g:32 * g + 1, t * N:(t + 1) * N],
                            Ofp[:, g:g + 1], sq[:],
                            start=True, stop=True,
                            tile_position=(0, 32 * g),
                        )
                nc.vector.tensor_scalar(
                    RFr[:], rp[:], -1.0 / 16.0, None, mybir.AluOpType.mult,
                )

            # ---- phase 2: per-feature gram + fixup + relu-sum cert ----
            # bank cols [0:256) = P[i in B0, all j]; cols [256:384) =
            # P[i in B1, j in B1] (the (B1,B0) block is the transpose of
            # (B0,B1), already covered by h=0).
            gp_ctx = tc.tile_pool(name="gpsum", bufs=6,
                                  space=bass.MemorySpace.PSUM)
            gpp = gp_ctx.__enter__()
            for f in range(F_LOC):
                t, a = divmod(f, NT)
                bank = gpp.tile([128, BANK_W], F32, tag="gram")
                ms = Mts[t][32 * a:32 * a + 32, :]
                for h, (j0, w) in enumerate(((0, 2 * 128), (128, 128))):
                    seg = bank[:, 128 * (2 * h):128 * (2 * h) + w]
                    nc.tensor.matmul(
                        seg,
                        Mts[t][32 * a:32 * a + 32,
                               h * 128:h * 128 + 128],
                        Mts[t][32 * a:32 * a + 32, j0:j0 + w],
                        start=True,
                        stop=False,
                        tile_position=(32 * a, 0),
                    )
                    nc.tensor.matmul(
                        seg,
                        RFr[32 * a:32 * a + 1,
                            t * N + h * 128:t * N + h * 128 + 128],
                        E8[32 * a:32 * a + 1, 0:j0 + w - j0],
                        start=False,
                        stop=False,
                        tile_position=(32 * a, 0),
                    )
                    nc.tensor.matmul(
                        seg,
                        E8[32 * a:32 * a + 1, 0:128],
                        RFr[32 * a:32 * a + 1, t * N + j0:t * N + j0 + w],
                        start=False,
                        stop=True,
                        tile_position=(32 * a, 0),
                    )
                if f % 3 != 2:
                    sa = scp.tile([128, BANK_W], BF16, tag="scrA")
                    nc.scalar.activation(
                        sa[:],
                        bank[:],
                        mybir.ActivationFunctionType.Relu,
                        bias=bth[:],
                        scale=1.0,
                        accum_out=certA[:, f:f + 1],
                    )
                else:
                    sd = scp.tile([128, BANK_W], BF16, tag="scrD")
                    nc.vector.tensor_scalar(
                        sd[:], bank[:], TH / 2, 0.0,
                        mybir.AluOpType.add, mybir.AluOpType.max,
                    )
                    nc.vector.tensor_reduce(
                        certD[:, f:f + 1], sd[:],
                        mybir.AxisListType.X, mybir.AluOpType.add,
                    )

            nc.sync.dma_start(certA_d[:], certA[:])
            nc.sync.dma_start(certD_d[:], certD[:])
            gp_ctx.__exit__(None, None, None)

    nc.compile()
    return nc


def _get_nc():
    if "nc" not in _CACHE:
        _CACHE["nc"] = _build()
    return _CACHE["nc"]


def _prep_inputs(x, T):
    x = np.asarray(x, dtype=np.float32)
    T = np.asarray(T, dtype=np.float32)
    xT = np.ascontiguousarray(x.T).astype(ml_dtypes.bfloat16)     # [2048, 256]
    xP = xT.reshape(NCT, 128, N).transpose(1, 0, 2)               # [128,16,256]
    Ofp = np.zeros((128, NT), dtype=ml_dtypes.bfloat16)
    idx = np.arange(128)
    Ofp[idx, idx // KD] = 1.0
    shared = {"Ofp": Ofp}
    for q in range(NQ):
        shared[f"xq{q}"] = np.ascontiguousarray(
            xP[:, 4 * q:4 * q + 4, :].reshape(128, 4 * N))
    in_maps = []
    for c in range(NCORES):
        f0 = c * F_LOC
        Tsl = T[:, f0:f0 + F_LOC, :].reshape(IN_F, FK).astype(ml_dtypes.bfloat16)
        TP = Tsl.reshape(NCT, 128, FK).transpose(1, 0, 2)         # [128,16,512]
        m = dict(shared)
        for q in range(NQ):
            m[f"tq{q}"] = np.ascontiguousarray(
                TP[:, 4 * q:4 * q + 4, :].reshape(128, 4 * FK))
        in_maps.append(m)
    return x, T, in_maps


def _exact_o_b(x, T):
    """Exact f32 o_b, matching the reference's underflow behavior."""
    M = (x @ T.reshape(IN_F, OUT_F * KD)).reshape(N, OUT_F, KD)
    o_b = np.zeros((N, OUT_F), dtype=np.float32)
    for i0 in range(0, N, 32):
        d = np.abs(M[i0:i0 + 32, None, :, :] - M[None, :, :, :]).sum(
            axis=3, dtype=np.float32
        )
        o_b += np.exp(-d.astype(np.float32)).sum(axis=0, dtype=np.float32)
    return o_b


def _run(x, T, trace=False):
    nc = _get_nc()
    x, T, in_maps = _prep_inputs(x, T)
    res = run_bass_kernel_spmd(nc, in_maps, core_ids=list(range(NCORES)), trace=trace)
    fsel = np.arange(F_LOC)
    acols = fsel[fsel % 3 != 2]
    dcols = fsel[fsel % 3 == 2]
    flagged = False
    for c in range(NCORES):
        certA = res.results[c]["certA"]                 # [128, F_LOC]
        certD = res.results[c]["certD"]
        if np.abs(certA[:, acols] - TH).max() > BAND:
            flagged = True
        if np.abs(certD[:, dcols] - TH).max() > BAND:
            flagged = True
    if flagged:
        o_b = _exact_o_b(x, T)
    else:
        o_b = np.ones((N, OUT_F), dtype=np.float32)
    out = np.concatenate([x, o_b], axis=1)
    return out, res


def kernel(x, T):
    out, _ = _run(x, T, trace=False)
    return out
